# revision 9
# baseline (speedup 1.0000x reference)
"""Trainium2 Bass kernel for RecursiveMamba130M.

Math: the complex SSM state telescopes to y_i = sum_{k<=i} G_{i-k} (.) u_k
with G_m[f] = sum_s Re(Cc R^m Bc).  Both projections are linear, so the
whole per-loop GEMM pair collapses into precomputed 768x768 matrices

    M_m = W_in^T @ (G_m[:,None] * out_proj^T),   z_i = sum_{k<=i} h_k @ M_{i-k}

(10 GEMM terms over the 4 loops instead of 8 big 768x1536 GEMMs, no
G-combine vector work, no yT transposes).

Sharding: data-parallel over the 1024 sequence positions (128 tokens per
core, no collectives); M_m replicated per core.

Device schedule (tokens on partitions, matmul path all bf16, norm sums
fp32):
  - h0 / h0^T / sum h0^2 precomputed on host; M0 DMAed in 128-row blocks
    and DMA issues spread over the SP and Act queues so z0 starts ~8us in.
  - z_i live in PSUM across loops; cross-loop terms h_k @ M_{j-k} fill
    the PE during each norm phase (keeps the p-state up).
  - rmsnorm via sum w^2 = rs_z^2 sum z^2 + 2 rs_z sum z.h + sum h^2;
    sum z^2 (Act) and sum 2zh (DVE) ping-pong over the two psum banks so
    they run concurrently; the [T,1] chain runs on GpSimd under the
    w = z*rs_z + h DVE op; h' halves pipeline into PE transpose, DVE
    psum->sbuf copies, and the final z-term.
"""

import numpy as np
import ml_dtypes

import concourse.tile as tile
from concourse.bacc import Bacc
from concourse import masks, mybir
from concourse.bass_utils import run_bass_kernel_spmd

T = 128          # tokens per core
D = 768          # d_model
KB = 6           # 128-blocks of d_model
NL = 4           # reasoning loops
NCORES = 8
EPS = 1e-6

f32 = mybir.dt.float32
bf16 = mybir.dt.bfloat16
AL = mybir.AluOpType
AF = mybir.ActivationFunctionType

_CACHE = {}


def build_nc():
    nc = Bacc()
    h0_d = nc.dram_tensor("h0", [T, D], bf16, kind="ExternalInput")
    h0T_d = nc.dram_tensor("h0T", [T, D], bf16, kind="ExternalInput")
    ssh0_d = nc.dram_tensor("ssh0", [T, 1], f32, kind="ExternalInput")
    m0_d = nc.dram_tensor("m0", [KB, 128, D], bf16, kind="ExternalInput")
    m_d = nc.dram_tensor("m4", [NL - 1, 128, KB * D], bf16,
                         kind="ExternalInput")
    sb_d = nc.dram_tensor("sb3", [NL - 1, T, D], bf16,
                           kind="ExternalInput")
    out_d = nc.dram_tensor("x_out", [T, D], f32, kind="ExternalOutput")

    with tile.TileContext(nc) as tc:
        with (
            tc.tile_pool(name="wpool", bufs=1) as wpool,
            tc.tile_pool(name="work", bufs=2) as work,
            tc.tile_pool(name="scal", bufs=1) as scal,
            tc.tile_pool(name="ps_z", bufs=1, space="PSUM") as ps_z,
            tc.tile_pool(name="ps_t", bufs=1, space="PSUM") as ps_t,
        ):
            # ---------- constants (issued before DMAs so the Act table
            # prewarm runs during the transfers) ----------
            ident = wpool.tile([128, 128], bf16, tag="ident")
            masks.make_identity(nc, ident[:])
            eps_t = wpool.tile([T, 1], f32, tag="eps_t")
            nc.vector.memset(eps_t[:], EPS)
            warm = scal.tile([T, 1], f32, tag="warm")
            nc.scalar.activation(warm[:], eps_t[:], AF.Sqrt,
                                 bias=eps_t[:, :], scale=1.0)

            # ---------- DMAs: availability-ordered, split across the two
            # hardware DGE queues (SP and Act) ----------
            hT = [None] * NL
            hT[0] = wpool.tile([T, D], bf16, tag="hT0", name="hT0")
            nc.sync.dma_start(hT[0][:], h0T_d[:, :])
            M0k = [None] * KB
            for k in (0, 2, 4):
                t = wpool.tile([128, D], bf16, tag=f"M0_{k}", name=f"M0_{k}")
                nc.sync.dma_start(t[:], m0_d[k])
                M0k[k] = t
            Mt = [None] * NL
            for m in (1, 2, 3):
                t = wpool.tile([128, KB * D], bf16, tag=f"M{m}", name=f"M{m}")
                nc.sync.dma_start(t[:], m_d[m - 1])
                Mt[m] = t
            # Act hardware-DGE queue: odd M0 blocks + h0 + ssh0
            t = wpool.tile([128, D], bf16, tag="M0_1", name="M0_1")
            nc.scalar.dma_start(t[:], m0_d[1])
            M0k[1] = t
            h0 = wpool.tile([T, D], bf16, tag="h0", name="h0")
            nc.scalar.dma_start(h0[:], h0_d[:, :])
            ssh0_t = scal.tile([T, 1], f32, tag="ssh0", name="ssh0")
            nc.scalar.dma_start(ssh0_t[:], ssh0_d[:, :])
            for k in (3, 5):
                t = wpool.tile([128, D], bf16, tag=f"M0_{k}", name=f"M0_{k}")
                nc.scalar.dma_start(t[:], m0_d[k])
                M0k[k] = t
            # gpsimd software-DGE: prebroadcast step-emb tiles
            Sb = [None] * NL
            for i in (1, 2, 3):
                sb = wpool.tile([T, D], bf16, tag=f"Sb{i}", name=f"Sb{i}")
                nc.gpsimd.dma_start(sb[:], sb_d[i - 1])
                Sb[i] = sb

            h = [None] * NL
            h[0] = h0
            ss_h = [None] * NL
            ss_h[0] = ssh0_t

            def zp_tile(j):
                return ps_z.tile([T, 1024], f32, tag=f"zp{j % 3}",
                                 name=f"z{j}")

            for i in (1, 2, 3):
                hT[i] = wpool.tile([T, D], bf16, tag=f"hT{i}", name=f"hT{i}")
                ss_h[i] = scal.tile([T, 1], f32, tag=f"ssh{i}",
                                    name=f"ssh{i}")

            def mblk(m, k):
                if m == 0:
                    return M0k[k][:, 0:D]
                return Mt[m][:, k * D:(k + 1) * D]

            def term(zt, i_h, m, start, stop, ks=range(KB)):
                for k in ks:
                    blk = mblk(m, k)
                    for off, nn in ((0, 512), (512, 256)):
                        nc.tensor.matmul(
                            zt[:, off:off + nn],
                            hT[i_h][:, 128 * k:128 * (k + 1)],
                            blk[:, off:off + nn],
                            start=(start and k == 0),
                            stop=(stop and k == KB - 1),
                        )

            def norm_adv(i, zt, prefetch=None):
                """Full per-loop tail: norm sums, rs factors, then either
                h_{i+1} (+transpose/copies, pipelined with the final term
                via caller) or the final output. `prefetch` is a callable
                issuing PE work that fills the norm gap."""
                last = i == NL - 1
                zA, zB = zt[:, 0:512], zt[:, 512:D]
                hA, hB = h[i][:, 0:512], h[i][:, 512:D]
                scrA = work.tile([T, D], bf16, tag="scrA", bufs=2)
                scrB = work.tile([T, D], bf16, tag="scrB", bufs=2)
                ss_zA = scal.tile([T, 1], f32, tag=f"sszA{i}")
                ss_zB = scal.tile([T, 1], f32, tag=f"sszB{i}")
                szhA = scal.tile([T, 1], f32, tag=f"szhA{i}")
                szhB = scal.tile([T, 1], f32, tag=f"szhB{i}")
                # ping-pong the two psum banks: Act on A while DVE on B,
                # then swap (read-read on one bank serializes)
                nc.scalar.activation(scrA[:, 0:512], zA, AF.Square,
                                     accum_out=ss_zA[:])
                nc.vector.scalar_tensor_tensor(
                    out=scrB[:, 512:D], in0=zB, scalar=2.0, in1=hB,
                    op0=AL.mult, op1=AL.mult, accum_out=szhB[:])
                nc.scalar.activation(scrA[:, 512:D], zB, AF.Square,
                                     accum_out=ss_zB[:])
                nc.vector.scalar_tensor_tensor(
                    out=scrB[:, 0:512], in0=zA, scalar=2.0, in1=hA,
                    op0=AL.mult, op1=AL.mult, accum_out=szhA[:])
                # [T,1] chain on gpsimd, off the DVE critical path
                ss_z = scal.tile([T, 1], f32, tag=f"ssz{i}")
                nc.gpsimd.tensor_add(ss_z[:], ss_zA[:], ss_zB[:])
                szh2 = scal.tile([T, 1], f32, tag=f"szh{i}")
                nc.gpsimd.tensor_add(szh2[:], szhA[:], szhB[:])
                sq_z = scal.tile([T, 1], f32, tag=f"sqz{i}")
                nc.scalar.activation(sq_z[:], ss_z[:], AF.Sqrt,
                                     bias=eps_t[:, :], scale=1.0 / D)
                rs_z = scal.tile([T, 1], f32, tag=f"rsz{i}")
                nc.vector.reciprocal(rs_z[:], sq_z[:])
                t1 = scal.tile([T, 1], f32, tag=f"t1_{i}")
                nc.vector.scalar_tensor_tensor(
                    out=t1[:], in0=ss_z[:], scalar=rs_z[:, :], in1=szh2[:],
                    op0=AL.mult, op1=AL.add)
                ss_w = scal.tile([T, 1], f32, tag=f"ssw{i}")
                nc.vector.scalar_tensor_tensor(
                    out=ss_w[:], in0=t1[:], scalar=rs_z[:, :], in1=ss_h[i][:],
                    op0=AL.mult, op1=AL.add)
                sq_w = scal.tile([T, 1], f32, tag=f"sqw{i}")
                nc.scalar.activation(sq_w[:], ss_w[:], AF.Sqrt,
                                     bias=eps_t[:, :], scale=1.0 / D)
                # w halves on DVE; rs_w recip lands between them
                w = work.tile([T, D], f32, tag="w", bufs=2)
                nc.vector.scalar_tensor_tensor(
                    out=w[:, 0:384], in0=zt[:, 0:384], scalar=rs_z[:, :],
                    in1=h[i][:, 0:384], op0=AL.mult, op1=AL.add)
                rs_w = scal.tile([T, 1], f32, tag=f"rsw{i}")
                nc.vector.reciprocal(rs_w[:], sq_w[:])
                nc.vector.scalar_tensor_tensor(
                    out=w[:, 384:D], in0=zt[:, 384:D], scalar=rs_z[:, :],
                    in1=h[i][:, 384:D], op0=AL.mult, op1=AL.add)
                if prefetch is not None:
                    prefetch()
                if last:
                    out_sb = wpool.tile([T, D], f32, tag="out_sb")
                    nc.vector.tensor_scalar_mul(out_sb[:, 0:384],
                                                w[:, 0:384], rs_w[:, :])
                    nc.sync.dma_start(out_d[:, 0:384], out_sb[:, 0:384])
                    nc.vector.tensor_scalar_mul(out_sb[:, 384:D],
                                                w[:, 384:D], rs_w[:, :])
                    nc.sync.dma_start(out_d[:, 384:D], out_sb[:, 384:D])
                    return
                # h_{i+1} in halves, pipelined into transpose + copies
                j = i + 1
                h[j] = wpool.tile([T, D], bf16, tag=f"h{j}", name=f"h{j}")
                nc.vector.scalar_tensor_tensor(
                    out=h[j][:, 0:384], in0=w[:, 0:384], scalar=rs_w[:, :],
                    in1=Sb[j][:, 0:384], op0=AL.mult, op1=AL.add)
                trp = ps_t.tile([T, 1024], bf16, tag="tr", name=f"tr{j}")
                for k in range(3):
                    nc.tensor.transpose(trp[:, 128 * k:128 * (k + 1)],
                                        h[j][:, 128 * k:128 * (k + 1)],
                                        ident[:])
                nc.vector.scalar_tensor_tensor(
                    out=h[j][:, 384:D], in0=w[:, 384:D], scalar=rs_w[:, :],
                    in1=Sb[j][:, 384:D], op0=AL.mult, op1=AL.add)
                nc.vector.tensor_copy(hT[j][:, 0:384], trp[:, 0:384])
                for k in range(3, KB):
                    nc.tensor.transpose(trp[:, 128 * k:128 * (k + 1)],
                                        h[j][:, 128 * k:128 * (k + 1)],
                                        ident[:])
                nc.vector.tensor_copy(hT[j][:, 384:D], trp[:, 384:D])
                scr = work.tile([T, D], bf16, tag="scrC", bufs=2,
                                name=f"scr{j}")
                nc.scalar.activation(scr[:], h[j][:], AF.Square,
                                     accum_out=ss_h[j][:])

            # ================= main pipeline =================
            z = [None] * NL
            z[0] = zp_tile(0)
            term(z[0], 0, 0, start=True, stop=True,
                 ks=(0, 2, 4, 1, 3, 5))

            z[1] = zp_tile(1)
            norm_adv(0, z[0],
                     prefetch=lambda: term(z[1], 0, 1, start=True,
                                           stop=False))
            term(z[1], 1, 0, start=False, stop=True, ks=range(0, 3))
            term(z[1], 1, 0, start=False, stop=True, ks=range(3, KB))

            z[2] = zp_tile(2)

            def g1():
                term(z[2], 0, 2, start=True, stop=False)
                term(z[2], 1, 1, start=False, stop=False)
            norm_adv(1, z[1], prefetch=g1)
            term(z[2], 2, 0, start=False, stop=True, ks=range(0, 3))
            term(z[2], 2, 0, start=False, stop=True, ks=range(3, KB))

            z[3] = zp_tile(3)

            def g2():
                term(z[3], 1, 2, start=True, stop=False)
                term(z[3], 2, 1, start=False, stop=False)
                term(z[3], 0, 3, start=False, stop=False)
            norm_adv(2, z[2], prefetch=g2)
            term(z[3], 3, 0, start=False, stop=True, ks=range(0, 3))
            term(z[3], 3, 0, start=False, stop=True, ks=range(3, KB))

            norm_adv(3, z[3])

    nc.compile()
    return nc


def _host_prep(x, in_proj_base, lora_A, lora_B, A_theta, B_real, B_imag,
               C_real, C_imag, out_proj_w, step_emb):
    W_in = in_proj_base.astype(np.float64) + 2.0 * (
        lora_B.astype(np.float64) @ lora_A.astype(np.float64))
    winT = W_in.T                                        # [768, 1536]
    woutT = out_proj_w.astype(np.float64).T              # [1536, 768]

    th = A_theta.astype(np.float64)
    P = (C_real.astype(np.float64) * B_real.astype(np.float64)
         - C_imag.astype(np.float64) * B_imag.astype(np.float64))
    Q = (C_real.astype(np.float64) * B_imag.astype(np.float64)
         + C_imag.astype(np.float64) * B_real.astype(np.float64))
    m_list = []
    for m in range(NL):
        g = (P * np.cos(m * th) - Q * np.sin(m * th)).sum(-1).reshape(-1)
        Mm = winT @ (g[:, None] * woutT)                 # [768, 768]
        # blocked layout: [partition, k*768+d] = Mm[k*128+partition, d]
        m_list.append(Mm.reshape(KB, 128, D).transpose(1, 0, 2)
                      .reshape(128, KB * D))
    mstack = np.stack(m_list).astype(ml_dtypes.bfloat16)
    m0 = np.ascontiguousarray(
        mstack[0].reshape(128, KB, D).transpose(1, 0, 2))   # [KB,128,D]
    m4 = np.ascontiguousarray(mstack[1:])                   # [3,128,KB*D]
    sb3 = np.ascontiguousarray(np.broadcast_to(
        step_emb[1:, None, :], (NL - 1, T, D))).astype(ml_dtypes.bfloat16)

    # h0 = x + step_emb[0], rounded to bf16 exactly as the device would use
    h0 = (x[0].astype(np.float64)
          + step_emb[0].astype(np.float64)).astype(ml_dtypes.bfloat16)
    h0f = h0.astype(np.float32)
    ssh0 = (h0f * h0f).sum(-1, keepdims=True).astype(np.float32)  # [L,1]
    return m0, m4, sb3, h0, ssh0


def kernel(x, in_proj_base, lora_A, lora_B, A_theta, B_real, B_imag,
           C_real, C_imag, out_proj_w, mixer_norm_w, loop_norm_w, step_emb,
           _trace=False):
    x = np.asarray(x, dtype=np.float32)
    m0, m4, sb3, h0_full, ssh0_full = _host_prep(
        x, np.asarray(in_proj_base), np.asarray(lora_A), np.asarray(lora_B),
        np.asarray(A_theta), np.asarray(B_real), np.asarray(B_imag),
        np.asarray(C_real), np.asarray(C_imag), np.asarray(out_proj_w),
        np.asarray(step_emb))
    # mixer_norm_w / loop_norm_w are ones per the problem spec; rmsnorm weight
    # multiplies are identity and omitted on device.

    if "nc" not in _CACHE:
        _CACHE["nc"] = build_nc()
    nc = _CACHE["nc"]

    shared = {"m0": m0, "m4": m4, "sb3": sb3}
    in_maps = []
    for c in range(NCORES):
        h0c = h0_full[T * c:T * (c + 1)]                       # [T, D] bf16
        h0Tc = np.ascontiguousarray(
            h0c.reshape(T, KB, 128).transpose(1, 2, 0))        # [KB,128,T]
        h0T_tile = np.ascontiguousarray(
            h0Tc.transpose(1, 0, 2).reshape(128, KB * T))
        in_maps.append({
            **shared,
            "h0": np.ascontiguousarray(h0c),
            "h0T": h0T_tile,
            "ssh0": np.ascontiguousarray(ssh0_full[T * c:T * (c + 1)]),
        })
    res = run_bass_kernel_spmd(nc, in_maps, list(range(NCORES)), trace=_trace)
    out = np.concatenate(
        [np.asarray(res.results[c]["x_out"]) for c in range(NCORES)], axis=0)
    if _trace:
        _CACHE["last_result"] = res
    return out[None, :, :].astype(np.float32)
